# revision 12
# baseline (speedup 1.0000x reference)
"""Multi-head attention (B=4, S=2048, D=1024, H=16) on 8 Trainium2 cores.

Sharding: (batch, head-group) grid — core c handles batch c//2, heads
(c%2)*8..(c%2)*8+8. Zero duplicated FLOPs; host sums the two partial
out-projections per batch and adds bo.

Per-core kernel (all fp32, matmuls in float32r = FP22 1-pass):
  phase A: K^T/Q^T [512,2048] (feature-major) + V [2048,520] (token-major,
           ones-augmented per head) projections from host-pre-transposed X^T.
  phase B: per (head-pair, q-block of 1024):
           S^T[k,q] = K·Q^T via row-packed C=64 matmuls (2 heads concurrent),
           P^T = exp(S^T/8) on ScalarE straight out of PSUM,
           AO^T_aug[65,q] += V_aug^T·P^T accumulated over 16 k-tiles
           (row 64 = softmax denominator), then normalize via reciprocal +
           gpsimd partition_broadcast.
  phase C: out-proj per q-block, token-major [2048,1024] partial to HBM.
"""

import numpy as np

import bass_rust
import concourse.bass as bass
import concourse.tile as tile
from concourse import mybir

F32 = mybir.dt.float32
F32R = mybir.dt.float32r

B, S, D = 4, 2048, 1024
NH, DK = 16, 64          # total heads, head dim
HG = 8                   # heads per core (head group)
DHG = HG * DK            # 512 features per head group
NP = 4                   # pairs of heads per core
QB = 1024                # q-block size
NQB = S // QB            # 2
KT = S // 128            # 16 k-tiles
CT = D // 128            # 8 contraction chunks for projections
VW = DK + 1              # 65: V columns per head incl. ones column


def split_multi_waits(nc):
    """This toolchain's walrus accepts only ONE sync-wait per instruction;
    Tile attaches several (one per producer proc). Hoist all but one wait
    onto single-wait NOPs inserted just before the instruction on the same
    engine (engines are in-order, so semantics are identical)."""
    uid = 0
    for f in nc.m.functions:
        for bb in f.blocks:
            il = bb.instructions
            i = 0
            while i < len(il):
                inst = il[i]
                si = inst.sync_info
                if si is not None and len(si.on_wait) > 1:
                    waits = list(si.on_wait)
                    inst.sync_info = bass_rust.SyncInfo(
                        on_wait=[waits[-1]], on_update=list(si.on_update)
                    )
                    for w in waits[:-1]:
                        nop = mybir.InstNoOp(
                            name=f"WSPLIT-{uid}",
                            engine=inst.engine,
                            bass_nofuse=True,
                            sync_info=bass_rust.SyncInfo(
                                on_wait=[w], on_update=[]
                            ),
                        )
                        uid += 1
                        il.insert(i, nop)
                        i += 1
                i += 1


def r(ap):
    return ap


def build_kernel():
    nc = bass.Bass(trn_type="TRN2")

    xq = nc.dram_tensor("xq", (D, S), F32R, kind="ExternalInput")   # query[b].T
    xk = nc.dram_tensor("xk", (D, S), F32R, kind="ExternalInput")
    xv = nc.dram_tensor("xv", (D, S), F32R, kind="ExternalInput")
    wq = nc.dram_tensor("wq", (D, DHG), F32R, kind="ExternalInput")  # Wq[hg].T
    wk = nc.dram_tensor("wk", (D, DHG), F32R, kind="ExternalInput")
    wv = nc.dram_tensor("wv", (D, DHG), F32R, kind="ExternalInput")
    wo = nc.dram_tensor("wo", (DHG, D), F32R, kind="ExternalInput")  # Wo[:,hg].T
    bq = nc.dram_tensor("bq", (DHG,), F32, kind="ExternalInput")
    bk = nc.dram_tensor("bk", (DHG,), F32, kind="ExternalInput")
    bv = nc.dram_tensor("bv", (DHG,), F32, kind="ExternalInput")
    out = nc.dram_tensor("out", (S, D), F32, kind="ExternalOutput")

    from contextlib import ExitStack

    with tile.TileContext(nc) as tc, ExitStack() as ctx:
        persist = ctx.enter_context(tc.tile_pool(name="persist", bufs=1))
        KT_sb = persist.tile([128, NP, S], F32R)        # K^T: pair p rows
        QT_sb = persist.tile([128, NP, S], F32R)        # Q^T
        V_sb = persist.tile([128, KT, HG, VW], F32R)    # V token-major + ones col
        AON = persist.tile([128, NP, S], F32R)          # normalized AO^T
        wo_sb = persist.tile([128, NP, D], F32R)        # out-proj weights by chunk
        bq_sb = persist.tile([128, NP], F32)
        bk_sb = persist.tile([128, NP], F32)
        bv_bc = persist.tile([128, DHG], F32)          # bv broadcast along parts

        nc.sync.dma_start(wo_sb[:], wo.rearrange("(c p) n -> p c n", p=128))
        with nc.allow_non_contiguous_dma(reason="tiny bias loads"):
            nc.sync.dma_start(bq_sb[:], bq.rearrange("(t p) -> p t", p=128))
            nc.sync.dma_start(bk_sb[:], bk.rearrange("(t p) -> p t", p=128))
        bvap = bv[:]
        nc.sync.dma_start(
            bv_bc[:],
            bass.AP(tensor=bvap.tensor, offset=bvap.offset, ap=[[0, 128], [1, DHG]]),
        )
        nc.vector.memset(V_sb[:, :, :, DK].bitcast(F32), 1.0)       # ones columns

        # ---- phase A: projections ------------------------------------------
        with tc.tile_pool(name="xstream", bufs=3) as xpool:
            # K^T / Q^T: out[dout 128, q] += w[:,ct,jt].T @ x^T[ct, q]
            with (
                tc.tile_pool(name="wkq", bufs=1) as wpool,
                tc.tile_pool(name="pproj", bufs=1, space="PSUM") as pproj,
            ):
                wk_sb = wpool.tile([128, CT, DHG], F32R)
                wq_sb = wpool.tile([128, CT, DHG], F32R)
                nc.sync.dma_start(wk_sb[:], wk.rearrange("(c p) n -> p c n", p=128))
                nc.sync.dma_start(wq_sb[:], wq.rearrange("(c p) n -> p c n", p=128))
                for xdram, w_sb, dst, b_sb in (
                    (xk, wk_sb, KT_sb, bk_sb),
                    (xq, wq_sb, QT_sb, bq_sb),
                ):
                    for kh in range(2):  # halves of the token dim
                        ps = [
                            pproj.tile([128, QB], F32, tag=f"proj{jt}",
                                       name=f"proj{jt}")
                            for jt in range(NP)
                        ]
                        for ct in range(CT):
                            xc = xpool.tile([128, QB], F32R, tag="xchunk")
                            nc.sync.dma_start(
                                xc[:], xdram[ct * 128:(ct + 1) * 128,
                                             kh * QB:(kh + 1) * QB]
                            )
                            for jt in range(NP):
                                for qc in range(QB // 512):
                                    nc.tensor.matmul(
                                        ps[jt][:, qc * 512:(qc + 1) * 512],
                                        r(w_sb[:, ct, jt * 128:(jt + 1) * 128]),
                                        r(xc[:, qc * 512:(qc + 1) * 512]),
                                        start=(ct == 0), stop=(ct == CT - 1),
                                    )
                        for jt in range(NP):
                            nc.vector.tensor_scalar_add(
                                dst[:, jt, kh * QB:(kh + 1) * QB],
                                ps[jt][:],
                                b_sb[:, jt:jt + 1],
                            )

            # V: out[tok 128, dv 512] += x^T[ct, tok].T @ w[:, ct, :]
            with (
                tc.tile_pool(name="wv", bufs=1) as wvpool,
                tc.tile_pool(name="pvproj", bufs=1, space="PSUM") as pvproj,
            ):
                wv_sb = wvpool.tile([128, CT, DHG], F32R)
                nc.sync.dma_start(wv_sb[:], wv.rearrange("(c p) n -> p c n", p=128))
                for th in range(2):  # halves of the token dim
                    ps = [
                        pvproj.tile([128, DHG], F32, tag=f"vproj{tt}",
                                    name=f"vproj{tt}")
                        for tt in range(8)
                    ]
                    for ct in range(CT):
                        xc = xpool.tile([128, QB], F32R, tag="xchunk")
                        nc.sync.dma_start(
                            xc[:], xv[ct * 128:(ct + 1) * 128,
                                      th * QB:(th + 1) * QB]
                        )
                        for tt in range(8):
                            nc.tensor.matmul(
                                ps[tt][:],
                                r(xc[:, tt * 128:(tt + 1) * 128]),
                                r(wv_sb[:, ct, :]),
                                start=(ct == 0), stop=(ct == CT - 1),
                            )
                    for tt in range(8):
                        vtile = th * 8 + tt
                        nc.vector.tensor_add(
                            V_sb[:, vtile, :, 0:DK],
                            ps[tt][:].rearrange("p (h d) -> p h d", d=DK),
                            bv_bc[:].rearrange("p (h d) -> p h d", d=DK),
                        )

        # ---- phases B+C: attention + out-projection ------------------------
        with (
            tc.tile_pool(name="pmm", bufs=1, space="PSUM") as pmm,
            tc.tile_pool(name="ptile", bufs=4) as ptp,
            tc.tile_pool(name="norm", bufs=4) as npool,
            tc.tile_pool(name="ostage", bufs=3) as opool,
            tc.tile_pool(name="dscratch", bufs=2, space="DRAM") as dpool,
        ):
            def outproj_tile(qb, tt):
                """Emit out-projection for token tile tt of q-block qb."""
                q0 = qb * QB
                ot = opool.tile([128, D], F32, tag="ot", name="ot")
                po = pmm.tile([128, QB], F32, tag="st", name="po", bufs=2)
                for oh in range(2):
                    for ci in range(NP):
                        nc.tensor.matmul(
                            po[:, oh * 512:(oh + 1) * 512],
                            r(AON[:, ci, q0 + tt * 128:q0 + (tt + 1) * 128]),
                            r(wo_sb[:, ci, oh * 512:(oh + 1) * 512]),
                            start=(ci == 0), stop=(ci == NP - 1),
                        )
                nc.vector.tensor_copy(ot[:], po[:])
                nc.sync.dma_start(
                    out[q0 + tt * 128:q0 + (tt + 1) * 128, :], ot[:]
                )

            for qb in range(NQB):
                q0 = qb * QB
                for p in range(NP):
                    ao = [
                        pmm.tile([VW, QB], F32, tag=f"ao{h2}", name=f"ao{h2}")
                        for h2 in range(2)
                    ]
                    for kt in range(KT):
                        for h2 in range(2):
                            hh = 2 * p + h2
                            lo, hi = h2 * DK, h2 * DK + DK
                            st = pmm.tile([128, QB], F32, tag="st", name="st",
                                          bufs=2)
                            for qc in range(QB // 512):
                                nc.tensor.matmul(
                                    st[:, qc * 512:(qc + 1) * 512],
                                    r(KT_sb[lo:hi, p, kt * 128:(kt + 1) * 128]),
                                    r(QT_sb[lo:hi, p,
                                            q0 + qc * 512:q0 + (qc + 1) * 512]),
                                    start=True, stop=True,
                                )
                            pt = ptp.tile([128, QB], F32R, tag="pt", name="pt")
                            nc.scalar.activation(
                                pt[:], st[:],
                                mybir.ActivationFunctionType.Exp,
                                scale=0.125,
                            )
                            for qc in range(QB // 512):
                                nc.tensor.matmul(
                                    ao[h2][:, qc * 512:(qc + 1) * 512],
                                    r(V_sb[:, kt, hh, :]),
                                    r(pt[:, qc * 512:(qc + 1) * 512]),
                                    start=(kt == 0), stop=(kt == KT - 1),
                                )
                    for h2 in range(2):
                        recip = npool.tile([1, QB], F32, tag="recip",
                                           name="recip")
                        nc.vector.reciprocal(recip[:], ao[h2][DK:VW, :])
                        dtmp = dpool.tile([1, QB], F32, tag="dtmp", name="dtmp")
                        nc.sync.dma_start(dtmp[:], recip[:])
                        rb = npool.tile([DK, QB], F32, tag="rb", name="rb")
                        dt_ap = dtmp[:]
                        nc.sync.dma_start(
                            rb[:],
                            bass.AP(tensor=dt_ap.tensor, offset=dt_ap.offset,
                                    ap=[[0, DK], [1, QB]]),
                        )
                        nc.vector.tensor_mul(
                            AON[h2 * DK:(h2 + 1) * DK, p, q0:q0 + QB],
                            ao[h2][0:DK, :],
                            rb[:],
                        )
                    # interleave previous q-block's out-projection
                    if qb > 0:
                        for tt in range(2 * p, 2 * p + 2):
                            outproj_tile(qb - 1, tt)
            for tt in range(QB // 128):
                outproj_tile(NQB - 1, tt)

    split_multi_waits(nc)
    return nc


def _prep_inputs(query, key, value, Wq, bq, Wk, bk, Wv, bv, Wo, bo):
    """Build the 8 per-core input maps."""
    xt = {}
    for nm, x in (("xq", query), ("xk", key), ("xv", value)):
        xt[nm] = [np.ascontiguousarray(x[b].T) for b in range(B)]
    in_maps = []
    for c in range(8):
        b, g = divmod(c, 2)
        rows = slice(g * DHG, (g + 1) * DHG)
        in_maps.append({
            "xq": xt["xq"][b], "xk": xt["xk"][b], "xv": xt["xv"][b],
            "wq": np.ascontiguousarray(Wq[rows, :].T),
            "wk": np.ascontiguousarray(Wk[rows, :].T),
            "wv": np.ascontiguousarray(Wv[rows, :].T),
            "wo": np.ascontiguousarray(Wo[:, rows].T),
            "bq": np.ascontiguousarray(bq[rows]),
            "bk": np.ascontiguousarray(bk[rows]),
            "bv": np.ascontiguousarray(bv[rows]),
        })
    return in_maps


_NC_CACHE = None


def run(inputs, trace=False):
    """Returns (full_output, BassKernelResults)."""
    global _NC_CACHE
    from concourse.bass_utils import run_bass_kernel_spmd

    inputs = {k: np.asarray(v, np.float32) for k, v in inputs.items()}
    in_maps = _prep_inputs(**inputs)
    if _NC_CACHE is None:
        _NC_CACHE = build_kernel()
    res = run_bass_kernel_spmd(
        _NC_CACHE, in_maps, core_ids=list(range(8)), trace=trace
    )
    bo = inputs["bo"]
    full = np.empty((B, S, D), np.float32)
    for b in range(B):
        full[b] = res.results[2 * b]["out"] + res.results[2 * b + 1]["out"] + bo
    return full, res


def kernel(**inputs):
    return run(inputs, trace=False)[0]
